# revision 1
# baseline (speedup 1.0000x reference)
"""Trainium2 Bass kernel for nn_Decoder_10110353014984.

Computation (see reference): hard-reset LIF over T=4 steps followed by a
linear head:
    v' = v + (x_t - v)/2 ; spike = (v' >= 1) ; v = (1-spike) * v'
    y  = einsum('tbnd,cd->tbnc', spikes, W) + b

The LIF is replicated with the reference's exact fp32 rounding order:
    d = x - v ; h = 0.5*d (exact) ; v' = v + h ; spike = v' >= 1 ;
    v = v' * (v' < 1)

Sharding: data-parallel over batch B=64 -> 8 per NeuronCore. The host
pre-transposes each shard to xT[T, D, S] (d-major) so LIF spike tiles are
directly the matmul stationary operand (no on-chip transposes), and
pre-transposes W to W^T[D, C] for the moving operand.

Matmul dtype: float32r (TF32-like, 1s/8e/11m, ~4x the fp32 matmul rate).
Spikes are {0,1} (exact in fp32r). Default: W rounded to fp32r on the host,
single pass -> rel err ~1.2e-4 (TF32-class), ~130us/run. KERNEL_HILO=1
splits W into fp32r-exact hi+lo parts (hi+lo == W exactly: 12+12 mantissa
bits) accumulated in one PSUM group -> fp32-exact result (~2e-7) at ~1.6x
the time. Bias is applied host-side (zeros in the spec).
"""

import os
import sys
import types

sys.path.insert(0, "/opt/trn_rl_repo")

import numpy as np

import concourse.bass as bass
import concourse.mybir as mybir
import concourse.tile as tile
from concourse.vector_clock import ScopedClock
import bass_rust as _br

T, B, N, D, C = 4, 64, 196, 512, 1000
NCORES = 8
BL = B // NCORES          # 8 batches per core
S = BL * N                # 1568 samples per timestep per core
P = 128                   # partition width
DCH = D // P              # 4 contraction tiles
SCH = (S + P - 1) // P    # 13 sample chunks (last has 32 rows)
CHALF = [(0, 500), (500, 500)]  # C split across two PSUM banks

F32 = mybir.dt.float32
F32R = mybir.dt.float32r
ALU = mybir.AluOpType


def round_fp32r(a):
    """Round fp32 -> fp32r (1s/8e/11m, RNE), matching walrus fp32_to_fp32r.
    Returns fp32 array whose values are exactly representable in fp32r."""
    u = np.ascontiguousarray(a, dtype=np.float32).view(np.uint32)
    lsb = (u >> np.uint32(12)) & np.uint32(1)
    u2 = u + np.uint32(0x7FF) + lsb          # round-to-nearest-even at bit 12
    u2 &= np.uint32(0xFFFFF000)
    return u2.view(np.float32)


def _patch_tile_drain():
    """This walrus build allows at most one sync wait per TPB_CTRL (Drain)
    instruction; Tile's tail drain carries one wait per active processor.
    Split it into a chain of single-wait drains (same-engine program order
    makes the conjunction equivalent)."""
    if getattr(tile.TileContext, "_drain_split_patch", False):
        return

    def _drain_and_barrier(self, tick_clock, wait_clock):
        drain_inst = self.nc.sync.drain()
        wait_clock.add_sem_waits(
            drain_inst.ins, ScopedClock({None: tick_clock.global_clock})
        )
        waits = (
            list(drain_inst.ins.sync_info.on_wait)
            if drain_inst.ins.has_wait()
            else []
        )
        if len(waits) > 1:
            drain_inst.ins.sync_info.on_wait = waits[:1]
            for i in range(1, len(waits)):
                d2 = self.nc.sync.drain()
                d2.ins.sync_info = _br.SyncInfo(on_wait=waits[i : i + 1], on_update=[])
        self.nc.all_engine_barrier()
        assert self.sems is not None
        popped = self.nc._tile_sem_poison_stack.pop()
        assert popped is self._sem_poison
        self.nc.clear_and_free_semaphores(list(self.sems.allocated().values()))
        self.nc.all_engine_barrier()

    tile.TileContext._drain_and_barrier = _drain_and_barrier

    # Same limit applies to every instruction class (Matmult, DMACopy, ...).
    # Before committing the scheduled instruction stream, shed all but one
    # wait per instruction onto standalone same-engine InstEventSemaphore
    # carriers placed immediately before it (engine program order preserves
    # the conjunction).
    _orig_lower = tile.TileContext._lower_ordered_insts

    def _split_lower(self, ordered):
        for bb_name, insts in ordered.items():
            new = []
            for inst in insts:
                si = inst.sync_info
                if si is not None and len(si.on_wait) > 1:
                    waits = list(si.on_wait)
                    for w in waits[:-1]:
                        ev = mybir.InstEventSemaphore(
                            name=self.nc.get_next_instruction_name(), ins=[], outs=[]
                        )
                        ev.engine = inst.engine
                        ev.sync_info = _br.SyncInfo(on_wait=[w], on_update=[])
                        new.append(ev)
                    inst.sync_info = _br.SyncInfo(
                        on_wait=[waits[-1]], on_update=list(si.on_update)
                    )
                new.append(inst)
            ordered[bb_name] = new
        return _orig_lower(self, ordered)

    tile.TileContext._lower_ordered_insts = _split_lower
    tile.TileContext._drain_split_patch = True


def _install_ntff_hook():
    """Register the axon NTFF profile hook missing from this image's antenv,
    so run_bass_kernel_spmd(trace=True) can report HW exec time."""
    if "antenv.axon_hooks" in sys.modules:
        return
    try:
        import antenv
        from trn_agent_boot.trn_boot import _ntff_profile_via_ctypes

        hook = _ntff_profile_via_ctypes("/opt/axon/libaxon_pjrt.so")
        mod = types.ModuleType("antenv.axon_hooks")
        mod.get_axon_ntff_profile_hook = lambda: hook
        mod.set_axon_ntff_profile_hook = lambda h: None
        sys.modules["antenv.axon_hooks"] = mod
        antenv.axon_hooks = mod
    except Exception:
        pass  # tracing degrades; execution still works


def build_nc(hilo=True):
    """One SPMD NeuronCore program; all 8 cores run it on their own shard."""
    _patch_tile_drain()
    nc = bass.Bass()
    xT = nc.dram_tensor("xT", [T, D, S], F32, kind="ExternalInput")
    whalves = [nc.dram_tensor("wT_hi", [D, C], F32R, kind="ExternalInput")]
    if hilo:
        whalves.append(nc.dram_tensor("wT_lo", [D, C], F32R, kind="ExternalInput"))
    y = nc.dram_tensor("y", [T, S, C], F32, kind="ExternalOutput")
    NH = len(whalves)

    with tile.TileContext(nc) as tc:
        with (
            tc.tile_pool(name="wpool", bufs=1) as wpool,
            tc.tile_pool(name="vpool", bufs=1) as vpool,
            tc.tile_pool(name="xpool", bufs=5) as xpool,
            tc.tile_pool(name="spool", bufs=2) as spool,
            tc.tile_pool(name="opool", bufs=6) as opool,
            tc.tile_pool(name="ppool", bufs=8, space="PSUM") as ppool,
        ):
            # Startup-critical ordering (subtile deps let consumers start on
            # partially-loaded tiles): first column-quarter of x(t=0) loads
            # first, then W (needed by the first matmul), then the rest of x0.
            QS = [(0, 384), (384, 384), (768, 384), (1152, S - 1152)]
            x0 = [xpool.tile([P, S], F32, tag="x", name=f"x0{d}") for d in range(DCH)]
            q0, qn = QS[0]
            for d in range(DCH):
                nc.sync.dma_start(
                    out=x0[d][:, q0 : q0 + qn],
                    in_=xT[0, d * P : (d + 1) * P, q0 : q0 + qn],
                )

            wt = [[None] * DCH for _ in range(NH)]
            for h in range(NH):
                for d in range(DCH):
                    w = wpool.tile([P, C], F32R, tag=f"w{h}{d}", name=f"w{h}{d}")
                    nc.sync.dma_start(out=w[:], in_=whalves[h][d * P : (d + 1) * P, :])
                    wt[h][d] = w

            for q0, qn in QS[1:]:
                for d in range(DCH):
                    nc.sync.dma_start(
                        out=x0[d][:, q0 : q0 + qn],
                        in_=xT[0, d * P : (d + 1) * P, q0 : q0 + qn],
                    )

            v = [None] * DCH
            xnext = x0
            for t in range(T):
                xcur, xnext = xnext, []
                sp = []
                if t == 0:
                    for d in range(DCH):
                        sp.append(
                            spool.tile([P, S], F32R, tag=f"sp{d}", name=f"sp{d}")
                        )
                        v[d] = vpool.tile([P, S], F32, tag=f"v{d}", name=f"v{d}")
                    for q0, qn in QS:
                        for d in range(DCH):
                            xq = xcur[d][:, q0 : q0 + qn]
                            sq = sp[d][:, q0 : q0 + qn]
                            # v' = 0.5*x (exact; matches v + (x-v)/2 with v=0)
                            nc.vector.tensor_scalar(
                                out=xq, in0=xq, scalar1=0.5, scalar2=None,
                                op0=ALU.mult,
                            )
                            nc.vector.tensor_scalar(
                                out=sq, in0=xq, scalar1=1.0, scalar2=None,
                                op0=ALU.is_ge,
                            )
                            nc.vector.scalar_tensor_tensor(
                                out=v[d][:, q0 : q0 + qn], in0=xq, scalar=1.0,
                                in1=xq, op0=ALU.is_lt, op1=ALU.mult,
                            )
                else:
                    for d in range(DCH):
                        xt = xcur[d]
                        # d = x - v, then v' = (d * 0.5) + v -- exact
                        # replication of the reference's rounding order
                        nc.vector.tensor_sub(xt[:], xt[:], v[d][:])
                        nc.vector.scalar_tensor_tensor(
                            out=xt[:], in0=xt[:], scalar=0.5, in1=v[d][:],
                            op0=ALU.mult, op1=ALU.add,
                        )
                        st = spool.tile([P, S], F32R, tag=f"sp{d}", name=f"sp{d}")
                        nc.vector.tensor_scalar(
                            out=st[:], in0=xt[:], scalar1=1.0, scalar2=None,
                            op0=ALU.is_ge,
                        )
                        sp.append(st)
                        if t < T - 1:
                            # v = (v' < 1) * v' (exact reset, spike in {0,1})
                            nc.vector.scalar_tensor_tensor(
                                out=v[d][:], in0=xt[:], scalar=1.0, in1=xt[:],
                                op0=ALU.is_lt, op1=ALU.mult,
                            )

                if t + 1 < T:
                    for d in range(DCH):
                        xt = xpool.tile([P, S], F32, tag="x", name=f"x{t+1}{d}")
                        nc.sync.dma_start(
                            out=xt[:], in_=xT[t + 1, d * P : (d + 1) * P, :]
                        )
                        xnext.append(xt)

                for k in range(SCH):
                    col0 = k * P
                    m = min(P, S - col0)
                    # one PSUM bank per C-half: finer-grained release lets the
                    # next chunk's matmuls start as soon as one bank is copied
                    ot = opool.tile([P, C], F32, tag="out")
                    for ci, (c0, cn) in enumerate(CHALF):
                        ps = ppool.tile([P, 512], F32, tag="ps")
                        for d in range(DCH):
                            lhsT = sp[d][:, col0 : col0 + m]
                            for h in range(NH):
                                nc.tensor.matmul(
                                    ps[:m, :cn],
                                    lhsT,
                                    wt[h][d][:, c0 : c0 + cn],
                                    start=(d == 0 and h == 0),
                                    stop=(d == DCH - 1 and h == NH - 1),
                                )
                        nc.scalar.copy(out=ot[:m, c0 : c0 + cn], in_=ps[:m, :cn])
                    nc.sync.dma_start(out=y[t, col0 : col0 + m, :], in_=ot[:m])
    return nc


_NC_CACHE = {}


def _get_nc(hilo=True):
    key = ("nc", hilo)
    if key not in _NC_CACHE:
        _NC_CACHE[key] = build_nc(hilo)
    return _NC_CACHE[key]


def _make_in_maps(x, W, hilo=True):
    WT = np.ascontiguousarray(W.T)  # [D, C]
    whi = round_fp32r(WT)
    maps_w = {"wT_hi": whi}
    if hilo:
        # residual is exact in fp32 (Sterbenz); round it to fp32r
        maps_w["wT_lo"] = round_fp32r(WT - whi)
    in_maps = []
    for c in range(NCORES):
        xc = x[:, c * BL : (c + 1) * BL].reshape(T, S, D)
        m = {"xT": np.ascontiguousarray(xc.transpose(0, 2, 1))}
        m.update(maps_w)
        in_maps.append(m)
    return in_maps


def kernel(x, W, b):
    from concourse.bass_utils import run_bass_kernel_spmd

    _install_ntff_hook()
    x = np.asarray(x, dtype=np.float32)
    W = np.asarray(W, dtype=np.float32)
    b = np.asarray(b, dtype=np.float32)

    # hilo=False (default): single fp32r pass, rel err ~1.2e-4 (TF32-class
    # matmul precision), ~130us. KERNEL_HILO=1: exact-fp32 hi+lo split
    # (rel err ~2e-7) at ~1.6x the time.
    hilo = os.environ.get("KERNEL_HILO", "0") == "1"
    nc = _get_nc(hilo)
    in_maps = _make_in_maps(x, W, hilo)
    res = run_bass_kernel_spmd(nc, in_maps, list(range(NCORES)))
    y = np.concatenate(
        [res.results[c]["y"].reshape(T, BL, N, C) for c in range(NCORES)], axis=1
    )
    if np.any(b):
        y = y + b[None, None, None, :]
    return np.ascontiguousarray(y, dtype=np.float32)



# revision 7
# speedup vs baseline: 1.0345x; 1.0345x over previous
"""Trainium2 Bass kernel for nn_Decoder_10110353014984.

Computation (see reference): hard-reset LIF over T=4 steps followed by a
linear head:
    v' = v + (x_t - v)/2 ; spike = (v' >= 1) ; v = (1-spike) * v'
    y  = einsum('tbnd,cd->tbnc', spikes, W) + b

Sharding: data-parallel over batch B=64 -> 8 per NeuronCore. The host
pre-transposes each shard to xT[T, D, S] (d-major) so LIF spike tiles are
directly the matmul stationary operand, and packs W^T (x16, split into
fp8e4m3 hi+lo parts) into DoubleRow pair layout.

LIF runs in u = 2*v space, which is bit-exact with the reference's fp32
rounding order (scaling by 2 commutes with RNE rounding):
    d = fl(x - u*0.5) == fl(x - v)
    u' = fl(u + d)    == 2*fl(v + d*0.5)
    spike = u' >= 2   == v' >= 1
    u = (u' < 2)*u'   == 2*(1-spike)*v'
The elementwise work is split across the DVE (vector) and Pool (gpsimd)
engines; spikes are written as fp8e4m3 {0,1}.

Matmul: fp8e4m3 DoubleRow perf mode (2 contraction rows/cycle, 2x the
bf16/fp32r rate). The two DoubleRow K-slots hold adjacent 128-row D-chunks;
W is pre-scaled by 16 and split into exact hi+lo fp8 parts (hi+lo == 16*W
to ~2^-8 relative), accumulated in the same PSUM group, so the full result
is ~bf16-accurate while streaming at fp8 speed. PSUM -> SBUF copy on the
Activation engine applies the 1/16 scale and downcasts to bf16; y returns
to HBM at 2 bytes/elem (halving output DMA traffic). Host upcasts to fp32.

KERNEL_FP8=0 falls back to the previous single-pass float32r kernel.
"""

import os
import sys
import types

sys.path.insert(0, "/opt/trn_rl_repo")

import numpy as np

import concourse.bass as bass
import concourse.mybir as mybir
import concourse.tile as tile
from concourse.vector_clock import ScopedClock
import bass_rust as _br

T, B, N, D, C = 4, 64, 196, 512, 1000
NCORES = 8
BL = B // NCORES          # 8 batches per core
S = BL * N                # 1568 samples per timestep per core
P = 128                   # partition width
DCH = D // P              # 4 contraction tiles
SCH = (S + P - 1) // P    # 13 sample chunks (last has 32 rows)
CP = 1024                 # C padded to 2 PSUM banks (512 fp32 each)
NPAIR = 2                 # DoubleRow d-chunk pairs: (d0,d1), (d2,d3)
WSCALE = 16.0             # W pre-scale keeps fp8 lo part out of subnormals

F32 = mybir.dt.float32
F32R = mybir.dt.float32r
BF16 = mybir.dt.bfloat16
F8 = mybir.dt.float8e4
ALU = mybir.AluOpType
DR = mybir.MatmulPerfMode.DoubleRow


def round_fp32r(a):
    """Round fp32 -> fp32r (1s/8e/11m, RNE), matching walrus fp32_to_fp32r.
    Returns fp32 array whose values are exactly representable in fp32r."""
    u = np.ascontiguousarray(a, dtype=np.float32).view(np.uint32)
    lsb = (u >> np.uint32(12)) & np.uint32(1)
    u2 = u + np.uint32(0x7FF) + lsb          # round-to-nearest-even at bit 12
    u2 &= np.uint32(0xFFFFF000)
    return u2.view(np.float32)


def _patch_tile_drain():
    """This walrus build allows at most one sync wait per TPB_CTRL (Drain)
    instruction; Tile's tail drain carries one wait per active processor.
    Split it into a chain of single-wait drains (same-engine program order
    makes the conjunction equivalent)."""
    if getattr(tile.TileContext, "_drain_split_patch", False):
        return

    def _drain_and_barrier(self, tick_clock, wait_clock):
        drain_inst = self.nc.sync.drain()
        wait_clock.add_sem_waits(
            drain_inst.ins, ScopedClock({None: tick_clock.global_clock})
        )
        waits = (
            list(drain_inst.ins.sync_info.on_wait)
            if drain_inst.ins.has_wait()
            else []
        )
        if len(waits) > 1:
            drain_inst.ins.sync_info.on_wait = waits[:1]
            for i in range(1, len(waits)):
                d2 = self.nc.sync.drain()
                d2.ins.sync_info = _br.SyncInfo(on_wait=waits[i : i + 1], on_update=[])
        self.nc.all_engine_barrier()
        assert self.sems is not None
        popped = self.nc._tile_sem_poison_stack.pop()
        assert popped is self._sem_poison
        self.nc.clear_and_free_semaphores(list(self.sems.allocated().values()))
        self.nc.all_engine_barrier()

    tile.TileContext._drain_and_barrier = _drain_and_barrier

    # Same limit applies to every instruction class (Matmult, DMACopy, ...).
    # Before committing the scheduled instruction stream, shed all but one
    # wait per instruction onto standalone same-engine InstEventSemaphore
    # carriers placed immediately before it (engine program order preserves
    # the conjunction).
    _orig_lower = tile.TileContext._lower_ordered_insts

    def _split_lower(self, ordered):
        for bb_name, insts in ordered.items():
            new = []
            for inst in insts:
                si = inst.sync_info
                if si is not None and len(si.on_wait) > 1:
                    waits = list(si.on_wait)
                    for w in waits[:-1]:
                        ev = mybir.InstEventSemaphore(
                            name=self.nc.get_next_instruction_name(), ins=[], outs=[]
                        )
                        ev.engine = inst.engine
                        ev.sync_info = _br.SyncInfo(on_wait=[w], on_update=[])
                        new.append(ev)
                    inst.sync_info = _br.SyncInfo(
                        on_wait=[waits[-1]], on_update=list(si.on_update)
                    )
                new.append(inst)
            ordered[bb_name] = new
        return _orig_lower(self, ordered)

    tile.TileContext._lower_ordered_insts = _split_lower
    tile.TileContext._drain_split_patch = True


def _install_ntff_hook():
    """Register the axon NTFF profile hook missing from this image's antenv,
    so run_bass_kernel_spmd(trace=True) can report HW exec time."""
    if "antenv.axon_hooks" in sys.modules:
        return
    try:
        import antenv
        from trn_agent_boot.trn_boot import _ntff_profile_via_ctypes

        hook = _ntff_profile_via_ctypes("/opt/axon/libaxon_pjrt.so")
        mod = types.ModuleType("antenv.axon_hooks")
        mod.get_axon_ntff_profile_hook = lambda: hook
        mod.set_axon_ntff_profile_hook = lambda h: None
        sys.modules["antenv.axon_hooks"] = mod
        antenv.axon_hooks = mod
    except Exception:
        pass  # tracing degrades; execution still works


def build_nc_fp8(sign_spike=False):
    """fp8 DoubleRow hi/lo kernel; one SPMD program for all 8 cores.

    sign_spike=True moves the spike threshold to the Activation engine as
    s' = Sign(u' - 2) in {-1, +1} (requires HW Sign(+0) == +1); the host
    then adds 0.5*colsum(W) since y = 0.5*S'@W + 0.5*colsum(W). The copy
    scale becomes 1/32. LIF charge/reset stay bit-exact on DVE.
    """
    _patch_tile_drain()
    nc = bass.Bass()
    xT = nc.dram_tensor("xT", [T, D, S], F32, kind="ExternalInput")
    wh = nc.dram_tensor("wh", [NPAIR, P, 2, CP], F8, kind="ExternalInput")
    wl = nc.dram_tensor("wl", [NPAIR, P, 2, CP], F8, kind="ExternalInput")
    # y packed as [T, 13, 128, C] bf16; chunk 12 uses rows 0:32 only.
    y16 = nc.dram_tensor("y16", [T, SCH, P, C], BF16, kind="ExternalOutput")
    OSCALE = 1.0 / (2.0 * WSCALE) if sign_spike else 1.0 / WSCALE
    SIGN = mybir.ActivationFunctionType.Sign

    with tile.TileContext(nc) as tc:
        with (
            tc.tile_pool(name="wpool", bufs=1) as wpool,
            tc.tile_pool(name="upool", bufs=1) as upool,
            tc.tile_pool(name="xpool", bufs=4) as xpool,
            tc.tile_pool(name="spool", bufs=2) as spool,
            tc.tile_pool(name="opool", bufs=2) as opool,
            tc.tile_pool(name="ppool", bufs=4, space="PSUM") as ppool,
        ):
            # All state lives in DoubleRow pair tiles [128, 2, S]: slot i of
            # pair p holds D-chunk 2p+i. Startup order: first column-quarter
            # of x(t=0), then W (needed by the first matmuls), then the rest
            # of x0. Subtile deps let the t0 LIF start on partial x tiles.
            QS = [(0, 384), (384, 384), (768, 384), (1152, S - 1152)]
            x0 = [
                xpool.tile([P, 2, S], F32, tag="x", name=f"x0{p}")
                for p in range(NPAIR)
            ]
            q0, qn = QS[0]
            for p in range(NPAIR):
                for sl in range(2):
                    r0 = (2 * p + sl) * P
                    nc.sync.dma_start(
                        out=x0[p][:, sl, q0 : q0 + qn],
                        in_=xT[0, r0 : r0 + P, q0 : q0 + qn],
                    )

            wt = {}
            for h, wsrc in (("h", wh), ("l", wl)):
                for p in range(NPAIR):
                    w = wpool.tile([P, 2, CP], F8, tag=f"w{h}{p}", name=f"w{h}{p}")
                    nc.sync.dma_start(out=w[:], in_=wsrc[p])
                    wt[h, p] = w

            for q0, qn in QS[1:]:
                for p in range(NPAIR):
                    for sl in range(2):
                        r0 = (2 * p + sl) * P
                        nc.sync.dma_start(
                            out=x0[p][:, sl, q0 : q0 + qn],
                            in_=xT[0, r0 : r0 + P, q0 : q0 + qn],
                        )

            u = [
                upool.tile([P, 2, S], F32, tag=f"u{p}", name=f"u{p}")
                for p in range(NPAIR)
            ]

            xnext = x0
            for t in range(T):
                xcur, xnext = xnext, []
                sp = [
                    spool.tile([P, 2, S], F8, tag=f"sp{p}", name=f"sp{p}_{t}")
                    for p in range(NPAIR)
                ]
                if t == 0:
                    # u0' = x exactly (v0' = x/2); spike = x >= 2;
                    # u0 = (x < 2) * x. Per-quarter so spikes stream out
                    # as soon as each x quarter lands.
                    for q0, qn in QS:
                        for p in range(NPAIR):
                            xq = xcur[p][:, :, q0 : q0 + qn]
                            sq = sp[p][:, :, q0 : q0 + qn]
                            if sign_spike:
                                nc.scalar.activation(
                                    out=sq, in_=xq, func=SIGN, bias=-2.0,
                                )
                            else:
                                nc.vector.tensor_scalar(
                                    out=sq, in0=xq, scalar1=2.0, scalar2=None,
                                    op0=ALU.is_ge,
                                )
                            nc.vector.scalar_tensor_tensor(
                                out=u[p][:, :, q0 : q0 + qn], in0=xq,
                                scalar=2.0, in1=xq,
                                op0=ALU.is_lt, op1=ALU.mult,
                            )
                else:
                    for p in range(NPAIR):
                        xt = xcur[p]
                        # d = fl(x - u*0.5); u' = fl(u + d); spike = u' >= 2;
                        # u = (u' < 2) * u'  (all bit-exact vs reference)
                        nc.vector.scalar_tensor_tensor(
                            out=xt[:], in0=u[p][:], scalar=-0.5, in1=xt[:],
                            op0=ALU.mult, op1=ALU.add,
                        )
                        nc.vector.tensor_add(u[p][:], u[p][:], xt[:])
                        if sign_spike:
                            nc.scalar.activation(
                                out=sp[p][:], in_=u[p][:], func=SIGN, bias=-2.0,
                            )
                        else:
                            nc.vector.tensor_scalar(
                                out=sp[p][:], in0=u[p][:],
                                scalar1=2.0, scalar2=None, op0=ALU.is_ge,
                            )
                        if t < T - 1:
                            nc.vector.scalar_tensor_tensor(
                                out=u[p][:], in0=u[p][:], scalar=2.0,
                                in1=u[p][:], op0=ALU.is_lt, op1=ALU.mult,
                            )

                if t + 1 < T:
                    for p in range(NPAIR):
                        xt = xpool.tile([P, 2, S], F32, tag="x", name=f"x{t+1}{p}")
                        for sl in range(2):
                            r0 = (2 * p + sl) * P
                            nc.sync.dma_start(
                                out=xt[:, sl, :],
                                in_=xT[t + 1, r0 : r0 + P, :],
                            )
                        xnext.append(xt)

                # 12 full chunks in 3 store-groups of 4, then the ragged 32.
                for g in range(3):
                    og = opool.tile([P, 4, C], BF16, tag="og", name=f"og{t}{g}")
                    for j in range(4):
                        k = 4 * g + j
                        col0 = k * P
                        ps = ppool.tile([P, CP], F32, tag="ps")
                        for p in range(NPAIR):
                            lhsT = sp[p][:, :, col0 : col0 + P]
                            for h in ("h", "l"):
                                for b in range(2):
                                    nc.tensor.matmul(
                                        ps[:, b * 512 : (b + 1) * 512],
                                        lhsT,
                                        wt[h, p][:, :, b * 512 : (b + 1) * 512],
                                        start=(p == 0 and h == "h"),
                                        stop=(p == NPAIR - 1 and h == "l"),
                                        perf_mode=DR,
                                    )
                        nc.scalar.mul(og[:, j, :], ps[:, 0:C], OSCALE)
                    nc.sync.dma_start(
                        out=y16[t, 4 * g : 4 * g + 4].transpose([1, 0, 2]),
                        in_=og[:],
                    )

                m = S - 12 * P  # 32
                col0 = 12 * P
                ps = ppool.tile([P, CP], F32, tag="ps")
                for p in range(NPAIR):
                    lhsT = sp[p][:, :, col0 : col0 + m]
                    for h in ("h", "l"):
                        for b in range(2):
                            nc.tensor.matmul(
                                ps[:m, b * 512 : (b + 1) * 512],
                                lhsT,
                                wt[h, p][:, :, b * 512 : (b + 1) * 512],
                                start=(p == 0 and h == "h"),
                                stop=(p == NPAIR - 1 and h == "l"),
                                perf_mode=DR,
                            )
                orr = opool.tile([P, C], BF16, tag="orr", name=f"orr{t}")
                nc.scalar.mul(orr[:m, :], ps[:m, 0:C], OSCALE)
                nc.sync.dma_start(out=y16[t, 12, :m, :], in_=orr[:m, :])
    return nc


def build_nc_fp32r(hilo=False):
    """Fallback: previous single/dual-pass float32r kernel."""
    _patch_tile_drain()
    nc = bass.Bass()
    xT = nc.dram_tensor("xT", [T, D, S], F32, kind="ExternalInput")
    whalves = [nc.dram_tensor("wT_hi", [D, C], F32R, kind="ExternalInput")]
    if hilo:
        whalves.append(nc.dram_tensor("wT_lo", [D, C], F32R, kind="ExternalInput"))
    y = nc.dram_tensor("y", [T, S, C], F32, kind="ExternalOutput")
    NH = len(whalves)
    CHALF = [(0, 500), (500, 500)]

    with tile.TileContext(nc) as tc:
        with (
            tc.tile_pool(name="wpool", bufs=1) as wpool,
            tc.tile_pool(name="vpool", bufs=1) as vpool,
            tc.tile_pool(name="xpool", bufs=5) as xpool,
            tc.tile_pool(name="spool", bufs=2) as spool,
            tc.tile_pool(name="opool", bufs=6) as opool,
            tc.tile_pool(name="ppool", bufs=8, space="PSUM") as ppool,
        ):
            QS = [(0, 384), (384, 384), (768, 384), (1152, S - 1152)]
            x0 = [xpool.tile([P, S], F32, tag="x", name=f"x0{d}") for d in range(DCH)]
            q0, qn = QS[0]
            for d in range(DCH):
                nc.sync.dma_start(
                    out=x0[d][:, q0 : q0 + qn],
                    in_=xT[0, d * P : (d + 1) * P, q0 : q0 + qn],
                )

            wt = [[None] * DCH for _ in range(NH)]
            for h in range(NH):
                for d in range(DCH):
                    w = wpool.tile([P, C], F32R, tag=f"w{h}{d}", name=f"w{h}{d}")
                    nc.sync.dma_start(out=w[:], in_=whalves[h][d * P : (d + 1) * P, :])
                    wt[h][d] = w

            for q0, qn in QS[1:]:
                for d in range(DCH):
                    nc.sync.dma_start(
                        out=x0[d][:, q0 : q0 + qn],
                        in_=xT[0, d * P : (d + 1) * P, q0 : q0 + qn],
                    )

            v = [None] * DCH
            xnext = x0
            for t in range(T):
                xcur, xnext = xnext, []
                sp = []
                if t == 0:
                    for d in range(DCH):
                        sp.append(
                            spool.tile([P, S], F32R, tag=f"sp{d}", name=f"sp{d}")
                        )
                        v[d] = vpool.tile([P, S], F32, tag=f"v{d}", name=f"v{d}")
                    for q0, qn in QS:
                        for d in range(DCH):
                            xq = xcur[d][:, q0 : q0 + qn]
                            sq = sp[d][:, q0 : q0 + qn]
                            nc.vector.tensor_scalar(
                                out=xq, in0=xq, scalar1=0.5, scalar2=None,
                                op0=ALU.mult,
                            )
                            nc.vector.tensor_scalar(
                                out=sq, in0=xq, scalar1=1.0, scalar2=None,
                                op0=ALU.is_ge,
                            )
                            nc.vector.scalar_tensor_tensor(
                                out=v[d][:, q0 : q0 + qn], in0=xq, scalar=1.0,
                                in1=xq, op0=ALU.is_lt, op1=ALU.mult,
                            )
                else:
                    for d in range(DCH):
                        xt = xcur[d]
                        nc.vector.tensor_sub(xt[:], xt[:], v[d][:])
                        nc.vector.scalar_tensor_tensor(
                            out=xt[:], in0=xt[:], scalar=0.5, in1=v[d][:],
                            op0=ALU.mult, op1=ALU.add,
                        )
                        st = spool.tile([P, S], F32R, tag=f"sp{d}", name=f"sp{d}")
                        nc.vector.tensor_scalar(
                            out=st[:], in0=xt[:], scalar1=1.0, scalar2=None,
                            op0=ALU.is_ge,
                        )
                        sp.append(st)
                        if t < T - 1:
                            nc.vector.scalar_tensor_tensor(
                                out=v[d][:], in0=xt[:], scalar=1.0, in1=xt[:],
                                op0=ALU.is_lt, op1=ALU.mult,
                            )

                if t + 1 < T:
                    for d in range(DCH):
                        xt = xpool.tile([P, S], F32, tag="x", name=f"x{t+1}{d}")
                        nc.sync.dma_start(
                            out=xt[:], in_=xT[t + 1, d * P : (d + 1) * P, :]
                        )
                        xnext.append(xt)

                for k in range(SCH):
                    col0 = k * P
                    m = min(P, S - col0)
                    ot = opool.tile([P, C], F32, tag="out")
                    for ci, (c0, cn) in enumerate(CHALF):
                        ps = ppool.tile([P, 512], F32, tag="ps")
                        for d in range(DCH):
                            lhsT = sp[d][:, col0 : col0 + m]
                            for h in range(NH):
                                nc.tensor.matmul(
                                    ps[:m, :cn],
                                    lhsT,
                                    wt[h][d][:, c0 : c0 + cn],
                                    start=(d == 0 and h == 0),
                                    stop=(d == DCH - 1 and h == NH - 1),
                                )
                        nc.scalar.copy(out=ot[:m, c0 : c0 + cn], in_=ps[:m, :cn])
                    nc.sync.dma_start(out=y[t, col0 : col0 + m, :], in_=ot[:m])
    return nc


_NC_CACHE = {}


def _get_nc(mode="fp8"):
    if mode not in _NC_CACHE:
        if mode == "fp8":
            _NC_CACHE[mode] = build_nc_fp8(sign_spike=False)
        elif mode == "fp8_sign":
            _NC_CACHE[mode] = build_nc_fp8(sign_spike=True)
        else:
            _NC_CACHE[mode] = build_nc_fp32r(hilo=(mode == "fp32r_hilo"))
    return _NC_CACHE[mode]


def _make_in_maps(x, W, mode="fp8"):
    WT = np.ascontiguousarray(W.T)  # [D, C]
    if mode.startswith("fp8"):
        f8 = mybir.dt.np(F8)
        wpad = np.zeros((D, CP), dtype=np.float32)
        wpad[:, :C] = WT * WSCALE
        whi8 = wpad.astype(f8)
        wlo8 = (wpad - whi8.astype(np.float32)).astype(f8)
        # [D, CP] -> [pair, 128, slot, CP] with slot = adjacent 128-row chunk
        def pack(a):
            return np.ascontiguousarray(
                a.reshape(NPAIR, 2, P, CP).transpose(0, 2, 1, 3)
            )
        maps_w = {"wh": pack(whi8), "wl": pack(wlo8)}
    else:
        whi = round_fp32r(WT)
        maps_w = {"wT_hi": whi}
        if mode == "fp32r_hilo":
            maps_w["wT_lo"] = round_fp32r(WT - whi)
    in_maps = []
    for c in range(NCORES):
        xc = x[:, c * BL : (c + 1) * BL].reshape(T, S, D)
        m = {"xT": np.ascontiguousarray(xc.transpose(0, 2, 1))}
        m.update(maps_w)
        in_maps.append(m)
    return in_maps


def kernel(x, W, b):
    from concourse.bass_utils import run_bass_kernel_spmd

    _install_ntff_hook()
    x = np.asarray(x, dtype=np.float32)
    W = np.asarray(W, dtype=np.float32)
    b = np.asarray(b, dtype=np.float32)

    mode = os.environ.get("KERNEL_MODE", "fp8")
    nc = _get_nc(mode)
    in_maps = _make_in_maps(x, W, mode)
    res = run_bass_kernel_spmd(nc, in_maps, list(range(NCORES)))
    bias = b.astype(np.float64)
    if mode.startswith("fp8"):
        parts = []
        for c in range(NCORES):
            yc = np.asarray(res.results[c]["y16"]).astype(np.float32)
            parts.append(yc.reshape(T, SCH * P, C)[:, :S].reshape(T, BL, N, C))
        y = np.concatenate(parts, axis=1)
        if mode == "fp8_sign":
            # y_dev = 0.5*S'@W with S' in {-1,+1}; add 0.5*colsum(W)
            bias = bias + 0.5 * W.astype(np.float64).sum(axis=1)
    else:
        y = np.concatenate(
            [res.results[c]["y"].reshape(T, BL, N, C) for c in range(NCORES)],
            axis=1,
        )
    if np.any(bias):
        y = y + bias.astype(np.float32)[None, None, None, :]
    return np.ascontiguousarray(y, dtype=np.float32)


# revision 9
# speedup vs baseline: 1.0854x; 1.0492x over previous
"""Trainium2 Bass kernel for nn_Decoder_10110353014984.

Computation (see reference): hard-reset LIF over T=4 steps followed by a
linear head:
    v' = v + (x_t - v)/2 ; spike = (v' >= 1) ; v = (1-spike) * v'
    y  = einsum('tbnd,cd->tbnc', spikes, W) + b

Sharding: data-parallel over batch B=64 -> 8 per NeuronCore. The host
pre-transposes each shard to xT[T, D, S] (d-major) so LIF spike tiles are
directly the matmul stationary operand, and packs W^T (x16, split into
fp8e4m3 hi+lo parts) into DoubleRow pair layout.

LIF runs in u = 2*v space, which is bit-exact with the reference's fp32
rounding order (scaling by 2 commutes with RNE rounding):
    d = fl(x - u*0.5) == fl(x - v)
    u' = fl(u + d)    == 2*fl(v + d*0.5)
    spike = u' >= 2   == v' >= 1
    u = (u' < 2)*u'   == 2*(1-spike)*v'
The elementwise work is split across the DVE (vector) and Pool (gpsimd)
engines; spikes are written as fp8e4m3 {0,1}.

Matmul: fp8e4m3 DoubleRow perf mode (2 contraction rows/cycle, 2x the
bf16/fp32r rate). The two DoubleRow K-slots hold adjacent 128-row D-chunks;
W is pre-scaled by 16 and split into exact hi+lo fp8 parts (hi+lo == 16*W
to ~2^-8 relative), accumulated in the same PSUM group, so the full result
is ~bf16-accurate while streaming at fp8 speed. PSUM -> SBUF copy on the
Activation engine applies the 1/16 scale and downcasts to bf16; y returns
to HBM at 2 bytes/elem (halving output DMA traffic). Host upcasts to fp32.

KERNEL_FP8=0 falls back to the previous single-pass float32r kernel.
"""

import os
import sys
import types

sys.path.insert(0, "/opt/trn_rl_repo")

import numpy as np

import concourse.bass as bass
import concourse.mybir as mybir
import concourse.tile as tile
from concourse.vector_clock import ScopedClock
import bass_rust as _br

T, B, N, D, C = 4, 64, 196, 512, 1000
NCORES = 8
BL = B // NCORES          # 8 batches per core
S = BL * N                # 1568 samples per timestep per core
P = 128                   # partition width
DCH = D // P              # 4 contraction tiles
SCH = (S + P - 1) // P    # 13 sample chunks (last has 32 rows)
CP = 1024                 # C padded to 2 PSUM banks (512 fp32 each)
NPAIR = 2                 # DoubleRow d-chunk pairs: (d0,d1), (d2,d3)
WSCALE = 16.0             # W pre-scale keeps fp8 lo part out of subnormals

F32 = mybir.dt.float32
F32R = mybir.dt.float32r
BF16 = mybir.dt.bfloat16
F8 = mybir.dt.float8e4
ALU = mybir.AluOpType
DR = mybir.MatmulPerfMode.DoubleRow


def round_fp32r(a):
    """Round fp32 -> fp32r (1s/8e/11m, RNE), matching walrus fp32_to_fp32r.
    Returns fp32 array whose values are exactly representable in fp32r."""
    u = np.ascontiguousarray(a, dtype=np.float32).view(np.uint32)
    lsb = (u >> np.uint32(12)) & np.uint32(1)
    u2 = u + np.uint32(0x7FF) + lsb          # round-to-nearest-even at bit 12
    u2 &= np.uint32(0xFFFFF000)
    return u2.view(np.float32)


def _patch_tile_drain():
    """This walrus build allows at most one sync wait per TPB_CTRL (Drain)
    instruction; Tile's tail drain carries one wait per active processor.
    Split it into a chain of single-wait drains (same-engine program order
    makes the conjunction equivalent)."""
    if getattr(tile.TileContext, "_drain_split_patch", False):
        return

    def _drain_and_barrier(self, tick_clock, wait_clock):
        drain_inst = self.nc.sync.drain()
        wait_clock.add_sem_waits(
            drain_inst.ins, ScopedClock({None: tick_clock.global_clock})
        )
        waits = (
            list(drain_inst.ins.sync_info.on_wait)
            if drain_inst.ins.has_wait()
            else []
        )
        if len(waits) > 1:
            drain_inst.ins.sync_info.on_wait = waits[:1]
            for i in range(1, len(waits)):
                d2 = self.nc.sync.drain()
                d2.ins.sync_info = _br.SyncInfo(on_wait=waits[i : i + 1], on_update=[])
        self.nc.all_engine_barrier()
        assert self.sems is not None
        popped = self.nc._tile_sem_poison_stack.pop()
        assert popped is self._sem_poison
        self.nc.clear_and_free_semaphores(list(self.sems.allocated().values()))
        self.nc.all_engine_barrier()

    tile.TileContext._drain_and_barrier = _drain_and_barrier

    # Same limit applies to every instruction class (Matmult, DMACopy, ...).
    # Before committing the scheduled instruction stream, shed all but one
    # wait per instruction onto standalone same-engine InstEventSemaphore
    # carriers placed immediately before it (engine program order preserves
    # the conjunction).
    _orig_lower = tile.TileContext._lower_ordered_insts

    def _split_lower(self, ordered):
        for bb_name, insts in ordered.items():
            new = []
            for inst in insts:
                si = inst.sync_info
                if si is not None and len(si.on_wait) > 1:
                    waits = list(si.on_wait)
                    for w in waits[:-1]:
                        ev = mybir.InstEventSemaphore(
                            name=self.nc.get_next_instruction_name(), ins=[], outs=[]
                        )
                        ev.engine = inst.engine
                        ev.sync_info = _br.SyncInfo(on_wait=[w], on_update=[])
                        new.append(ev)
                    inst.sync_info = _br.SyncInfo(
                        on_wait=[waits[-1]], on_update=list(si.on_update)
                    )
                new.append(inst)
            ordered[bb_name] = new
        return _orig_lower(self, ordered)

    tile.TileContext._lower_ordered_insts = _split_lower
    tile.TileContext._drain_split_patch = True


def _install_ntff_hook():
    """Register the axon NTFF profile hook missing from this image's antenv,
    so run_bass_kernel_spmd(trace=True) can report HW exec time."""
    if "antenv.axon_hooks" in sys.modules:
        return
    try:
        import antenv
        from trn_agent_boot.trn_boot import _ntff_profile_via_ctypes

        hook = _ntff_profile_via_ctypes("/opt/axon/libaxon_pjrt.so")
        mod = types.ModuleType("antenv.axon_hooks")
        mod.get_axon_ntff_profile_hook = lambda: hook
        mod.set_axon_ntff_profile_hook = lambda h: None
        sys.modules["antenv.axon_hooks"] = mod
        antenv.axon_hooks = mod
    except Exception:
        pass  # tracing degrades; execution still works


S2 = SCH * P              # 1664: S padded to whole 128-chunks
CH_HALVES = [(0, 7), (7, SCH)]          # LIF halves, chunk-aligned
CH_QUARTERS = [(0, 3), (3, 6), (6, 9), (9, SCH)]  # t0 quarters


def build_nc_fp8(sign_spike=False):
    """fp8 DoubleRow hi/lo kernel; one SPMD program for all 8 cores.

    All per-sample state (x, u, spikes) lives in chunk-major pair tiles
    [128, 13, 2, 128]: chunk k, DoubleRow slot s (= D-chunk 2p+s), sample
    within chunk. Every LIF write and every matmul lhsT read is then a
    contiguous SBUF range, so Tile's subtile dependency tracking lets
    matmuls for chunk k start as soon as the LIF half/quarter covering k
    is done. x is host-padded to 1664 samples (pad columns are zeros).

    sign_spike=True moves the spike threshold to the Activation engine as
    s' = Sign(u' - 2) in {-1, +1} (requires HW Sign(+0) == +1); the host
    then adds 0.5*colsum(W) since y = 0.5*S'@W + 0.5*colsum(W). The copy
    scale becomes 1/32. LIF charge/reset stay bit-exact on DVE.
    """
    _patch_tile_drain()
    nc = bass.Bass()
    xT = nc.dram_tensor("xT", [T, D, S2], F32, kind="ExternalInput")
    wh = nc.dram_tensor("wh", [NPAIR, P, 2, C], F8, kind="ExternalInput")
    wl = nc.dram_tensor("wl", [NPAIR, P, 2, C], F8, kind="ExternalInput")
    # y packed as [T, 13, 128, C] bf16; chunk 12 uses rows 0:32 only.
    y16 = nc.dram_tensor("y16", [T, SCH, P, C], BF16, kind="ExternalOutput")
    OSCALE = 1.0 / (2.0 * WSCALE) if sign_spike else 1.0 / WSCALE
    SIGN = mybir.ActivationFunctionType.Sign

    def lif_ops(t, xcur, u, sp, k0, k1):
        """LIF update over chunks [k0, k1) for both pairs; bit-exact vs ref."""
        for p in range(NPAIR):
            xs = xcur[p][:, k0:k1]
            us = u[p][:, k0:k1]
            ss = sp[p][:, k0:k1]
            if t == 0:
                # u0' = x exactly (v0' = x/2); spike = x >= 2
                if sign_spike:
                    nc.scalar.activation(out=ss, in_=xs, func=SIGN, bias=-2.0)
                else:
                    nc.vector.tensor_scalar(
                        out=ss, in0=xs, scalar1=2.0, scalar2=None, op0=ALU.is_ge
                    )
            else:
                # d = fl(x - u*0.5); u' = fl(u + d); spike = u' >= 2
                nc.vector.scalar_tensor_tensor(
                    out=xs, in0=us, scalar=-0.5, in1=xs,
                    op0=ALU.mult, op1=ALU.add,
                )
                nc.vector.tensor_add(us, us, xs)
                if sign_spike:
                    nc.scalar.activation(out=ss, in_=us, func=SIGN, bias=-2.0)
                else:
                    nc.vector.tensor_scalar(
                        out=ss, in0=us, scalar1=2.0, scalar2=None, op0=ALU.is_ge
                    )

    def reset_ops(t, xcur, u, k0, k1):
        """Hard reset: u = (u' < 2) * u' (t=0: u = (x < 2) * x)."""
        for p in range(NPAIR):
            src = xcur[p][:, k0:k1] if t == 0 else u[p][:, k0:k1]
            nc.vector.scalar_tensor_tensor(
                out=u[p][:, k0:k1], in0=src, scalar=2.0, in1=src,
                op0=ALU.is_lt, op1=ALU.mult,
            )

    with tile.TileContext(nc) as tc:
        with (
            tc.tile_pool(name="wpool", bufs=1) as wpool,
            tc.tile_pool(name="upool", bufs=1) as upool,
            tc.tile_pool(name="xpool", bufs=4) as xpool,
            tc.tile_pool(name="spool", bufs=2) as spool,
            tc.tile_pool(name="opool", bufs=4) as opool,
            tc.tile_pool(name="ppool", bufs=4, space="PSUM") as ppool,
        ):
            # W first (gates the first matmul group), then x0 by quarters.
            wt = {}
            for h, wsrc in (("h", wh), ("l", wl)):
                for p in range(NPAIR):
                    w = wpool.tile([P, 2, C], F8, tag=f"w{h}{p}", name=f"w{h}{p}")
                    nc.sync.dma_start(out=w[:], in_=wsrc[p])
                    wt[h, p] = w

            x0 = [
                xpool.tile([P, SCH, 2, P], F32, tag="x", name=f"x0{p}")
                for p in range(NPAIR)
            ]
            for k0, k1 in CH_QUARTERS:
                for p in range(NPAIR):
                    for sl in range(2):
                        r0 = (2 * p + sl) * P
                        nc.sync.dma_start(
                            out=x0[p][:, k0:k1, sl, :],
                            in_=xT[0, r0 : r0 + P, k0 * P : k1 * P],
                        )

            u = [
                upool.tile([P, SCH, 2, P], F32, tag=f"u{p}", name=f"u{p}")
                for p in range(NPAIR)
            ]

            xnext = x0
            for t in range(T):
                xcur, xnext = xnext, []
                sp = [
                    spool.tile([P, SCH, 2, P], F8, tag=f"sp{p}", name=f"sp{p}_{t}")
                    for p in range(NPAIR)
                ]
                # Spike passes first (they gate the matmuls), resets after.
                spans = CH_QUARTERS if t == 0 else CH_HALVES
                for k0, k1 in spans:
                    lif_ops(t, xcur, u, sp, k0, k1)
                if t < T - 1:
                    for k0, k1 in spans:
                        reset_ops(t, xcur, u, k0, k1)

                if t + 1 < T:
                    for p in range(NPAIR):
                        xt = xpool.tile(
                            [P, SCH, 2, P], F32, tag="x", name=f"x{t+1}{p}"
                        )
                        for k0, k1 in CH_HALVES:
                            for sl in range(2):
                                r0 = (2 * p + sl) * P
                                nc.sync.dma_start(
                                    out=xt[:, k0:k1, sl, :],
                                    in_=xT[t + 1, r0 : r0 + P, k0 * P : k1 * P],
                                )
                        xnext.append(xt)

                # 12 full chunks in 3 store-groups of 4, then the ragged 32.
                def chunk_matmuls(k, m):
                    ps = ppool.tile([P, 2, 512], F32, tag="ps")
                    for p in range(NPAIR):
                        lhsT = sp[p][:, k, :, 0:m] if m < P else sp[p][:, k]
                        for h in ("h", "l"):
                            for b in range(2):
                                nc.tensor.matmul(
                                    ps[:m, b, 0:500],
                                    lhsT,
                                    wt[h, p][:, :, b * 500 : (b + 1) * 500],
                                    start=(p == 0 and h == "h"),
                                    stop=(p == NPAIR - 1 and h == "l"),
                                    perf_mode=DR,
                                )
                    return ps

                for g in range(3):
                    og = opool.tile([P, 4, C], BF16, tag="og", name=f"og{t}{g}")
                    for j in range(4):
                        ps = chunk_matmuls(4 * g + j, P)
                        nc.scalar.mul(
                            og[:, j].rearrange("p (two c) -> p two c", two=2),
                            ps[:, :, 0:500],
                            OSCALE,
                        )
                    nc.sync.dma_start(
                        out=y16[t, 4 * g : 4 * g + 4].transpose([1, 0, 2]),
                        in_=og[:],
                    )

                m = S - 12 * P  # 32
                ps = chunk_matmuls(12, m)
                orr = opool.tile([P, C], BF16, tag="orr", name=f"orr{t}")
                nc.scalar.mul(
                    orr[:m].rearrange("p (two c) -> p two c", two=2),
                    ps[:m, :, 0:500],
                    OSCALE,
                )
                nc.sync.dma_start(out=y16[t, 12, :m, :], in_=orr[:m, :])
    return nc


def build_nc_fp32r(hilo=False):
    """Fallback: previous single/dual-pass float32r kernel."""
    _patch_tile_drain()
    nc = bass.Bass()
    xT = nc.dram_tensor("xT", [T, D, S], F32, kind="ExternalInput")
    whalves = [nc.dram_tensor("wT_hi", [D, C], F32R, kind="ExternalInput")]
    if hilo:
        whalves.append(nc.dram_tensor("wT_lo", [D, C], F32R, kind="ExternalInput"))
    y = nc.dram_tensor("y", [T, S, C], F32, kind="ExternalOutput")
    NH = len(whalves)
    CHALF = [(0, 500), (500, 500)]

    with tile.TileContext(nc) as tc:
        with (
            tc.tile_pool(name="wpool", bufs=1) as wpool,
            tc.tile_pool(name="vpool", bufs=1) as vpool,
            tc.tile_pool(name="xpool", bufs=5) as xpool,
            tc.tile_pool(name="spool", bufs=2) as spool,
            tc.tile_pool(name="opool", bufs=6) as opool,
            tc.tile_pool(name="ppool", bufs=8, space="PSUM") as ppool,
        ):
            QS = [(0, 384), (384, 384), (768, 384), (1152, S - 1152)]
            x0 = [xpool.tile([P, S], F32, tag="x", name=f"x0{d}") for d in range(DCH)]
            q0, qn = QS[0]
            for d in range(DCH):
                nc.sync.dma_start(
                    out=x0[d][:, q0 : q0 + qn],
                    in_=xT[0, d * P : (d + 1) * P, q0 : q0 + qn],
                )

            wt = [[None] * DCH for _ in range(NH)]
            for h in range(NH):
                for d in range(DCH):
                    w = wpool.tile([P, C], F32R, tag=f"w{h}{d}", name=f"w{h}{d}")
                    nc.sync.dma_start(out=w[:], in_=whalves[h][d * P : (d + 1) * P, :])
                    wt[h][d] = w

            for q0, qn in QS[1:]:
                for d in range(DCH):
                    nc.sync.dma_start(
                        out=x0[d][:, q0 : q0 + qn],
                        in_=xT[0, d * P : (d + 1) * P, q0 : q0 + qn],
                    )

            v = [None] * DCH
            xnext = x0
            for t in range(T):
                xcur, xnext = xnext, []
                sp = []
                if t == 0:
                    for d in range(DCH):
                        sp.append(
                            spool.tile([P, S], F32R, tag=f"sp{d}", name=f"sp{d}")
                        )
                        v[d] = vpool.tile([P, S], F32, tag=f"v{d}", name=f"v{d}")
                    for q0, qn in QS:
                        for d in range(DCH):
                            xq = xcur[d][:, q0 : q0 + qn]
                            sq = sp[d][:, q0 : q0 + qn]
                            nc.vector.tensor_scalar(
                                out=xq, in0=xq, scalar1=0.5, scalar2=None,
                                op0=ALU.mult,
                            )
                            nc.vector.tensor_scalar(
                                out=sq, in0=xq, scalar1=1.0, scalar2=None,
                                op0=ALU.is_ge,
                            )
                            nc.vector.scalar_tensor_tensor(
                                out=v[d][:, q0 : q0 + qn], in0=xq, scalar=1.0,
                                in1=xq, op0=ALU.is_lt, op1=ALU.mult,
                            )
                else:
                    for d in range(DCH):
                        xt = xcur[d]
                        nc.vector.tensor_sub(xt[:], xt[:], v[d][:])
                        nc.vector.scalar_tensor_tensor(
                            out=xt[:], in0=xt[:], scalar=0.5, in1=v[d][:],
                            op0=ALU.mult, op1=ALU.add,
                        )
                        st = spool.tile([P, S], F32R, tag=f"sp{d}", name=f"sp{d}")
                        nc.vector.tensor_scalar(
                            out=st[:], in0=xt[:], scalar1=1.0, scalar2=None,
                            op0=ALU.is_ge,
                        )
                        sp.append(st)
                        if t < T - 1:
                            nc.vector.scalar_tensor_tensor(
                                out=v[d][:], in0=xt[:], scalar=1.0, in1=xt[:],
                                op0=ALU.is_lt, op1=ALU.mult,
                            )

                if t + 1 < T:
                    for d in range(DCH):
                        xt = xpool.tile([P, S], F32, tag="x", name=f"x{t+1}{d}")
                        nc.sync.dma_start(
                            out=xt[:], in_=xT[t + 1, d * P : (d + 1) * P, :]
                        )
                        xnext.append(xt)

                for k in range(SCH):
                    col0 = k * P
                    m = min(P, S - col0)
                    ot = opool.tile([P, C], F32, tag="out")
                    for ci, (c0, cn) in enumerate(CHALF):
                        ps = ppool.tile([P, 512], F32, tag="ps")
                        for d in range(DCH):
                            lhsT = sp[d][:, col0 : col0 + m]
                            for h in range(NH):
                                nc.tensor.matmul(
                                    ps[:m, :cn],
                                    lhsT,
                                    wt[h][d][:, c0 : c0 + cn],
                                    start=(d == 0 and h == 0),
                                    stop=(d == DCH - 1 and h == NH - 1),
                                )
                        nc.scalar.copy(out=ot[:m, c0 : c0 + cn], in_=ps[:m, :cn])
                    nc.sync.dma_start(out=y[t, col0 : col0 + m, :], in_=ot[:m])
    return nc


_NC_CACHE = {}


def _get_nc(mode="fp8"):
    if mode not in _NC_CACHE:
        if mode == "fp8":
            _NC_CACHE[mode] = build_nc_fp8(sign_spike=False)
        elif mode == "fp8_sign":
            _NC_CACHE[mode] = build_nc_fp8(sign_spike=True)
        else:
            _NC_CACHE[mode] = build_nc_fp32r(hilo=(mode == "fp32r_hilo"))
    return _NC_CACHE[mode]


def _make_in_maps(x, W, mode="fp8"):
    WT = np.ascontiguousarray(W.T)  # [D, C]
    if mode.startswith("fp8"):
        f8 = mybir.dt.np(F8)
        w16 = WT * WSCALE
        whi8 = w16.astype(f8)
        wlo8 = (w16 - whi8.astype(np.float32)).astype(f8)
        # [D, C] -> [pair, 128, slot, C] with slot = adjacent 128-row chunk
        def pack(a):
            return np.ascontiguousarray(
                a.reshape(NPAIR, 2, P, C).transpose(0, 2, 1, 3)
            )
        maps_w = {"wh": pack(whi8), "wl": pack(wlo8)}
    else:
        whi = round_fp32r(WT)
        maps_w = {"wT_hi": whi}
        if mode == "fp32r_hilo":
            maps_w["wT_lo"] = round_fp32r(WT - whi)
    in_maps = []
    for c in range(NCORES):
        xc = x[:, c * BL : (c + 1) * BL].reshape(T, S, D)
        xt = np.ascontiguousarray(xc.transpose(0, 2, 1))  # [T, D, S]
        if mode.startswith("fp8"):
            xp = np.zeros((T, D, S2), dtype=np.float32)
            xp[:, :, :S] = xt
            xt = xp
        m = {"xT": xt}
        m.update(maps_w)
        in_maps.append(m)
    return in_maps


def kernel(x, W, b):
    from concourse.bass_utils import run_bass_kernel_spmd

    _install_ntff_hook()
    x = np.asarray(x, dtype=np.float32)
    W = np.asarray(W, dtype=np.float32)
    b = np.asarray(b, dtype=np.float32)

    mode = os.environ.get("KERNEL_MODE", "fp8")
    nc = _get_nc(mode)
    in_maps = _make_in_maps(x, W, mode)
    res = run_bass_kernel_spmd(nc, in_maps, list(range(NCORES)))
    bias = b.astype(np.float64)
    if mode.startswith("fp8"):
        parts = []
        for c in range(NCORES):
            yc = np.asarray(res.results[c]["y16"]).astype(np.float32)
            parts.append(yc.reshape(T, SCH * P, C)[:, :S].reshape(T, BL, N, C))
        y = np.concatenate(parts, axis=1)
        if mode == "fp8_sign":
            # y_dev = 0.5*S'@W with S' in {-1,+1}; add 0.5*colsum(W)
            bias = bias + 0.5 * W.astype(np.float64).sum(axis=1)
    else:
        y = np.concatenate(
            [res.results[c]["y"].reshape(T, BL, N, C) for c in range(NCORES)],
            axis=1,
        )
    if np.any(bias):
        y = y + bias.astype(np.float32)[None, None, None, :]
    return np.ascontiguousarray(y, dtype=np.float32)
